# revision 6
# baseline (speedup 1.0000x reference)
"""Trainium2 Bass kernel for nn_Circuit: batched 3-qubit circuit.

Circuit per state (8-dim complex, B=2^21 independent states):
  H on qubits 0,1 -> RX(theta0) on q0, RX(theta1) on q1 -> CNOT(q0 -> q2).
The whole circuit is one 8x8 complex unitary U(theta); the kernel applies
y = U @ x per state and emits (B, 8, 2) with real/imag stacked last.

Device strategy (pure data-parallel over 8 cores, B/8 states per core):
  - load natural-layout fp32 tiles [128 part x (W*8)] (W states per row,
    contiguous >=512B per-partition DMA chunks)
  - PE transpose 128x128 blocks to put the 8 state components on partitions
  - one pair of accumulating matmuls against host-built 128x256
    block-diagonal gate matrices BDr/BDi; the matmul output is already in
    natural layout with real/imag interleaved
  - copy PSUM->SBUF, contiguous DMA out.
"""

import os
import numpy as np

import concourse.bass as bass
import concourse.mybir as mybir
import concourse.tile as tile
from concourse import masks
from concourse.bass_utils import run_bass_kernel_spmd

F32 = mybir.dt.float32
F32R = mybir.dt.float32r

B = 2097152            # total batch
N_CORES = 8
S_CORE = B // N_CORES  # states per core = 262144
W = 64                 # states per partition-row per iteration
STATES_PER_ITER = 128 * W          # 8192
N_ITERS = S_CORE // STATES_PER_ITER  # 32

# set KERNEL_FP32R=0 to fall back to plain-fp32 matmuls
USE_FP32R = os.environ.get("KERNEL_FP32R", "1") == "1"


def circuit_unitary(theta):
    """8x8 complex64 unitary of the whole circuit, component index 4a+2b+c
    for qubits (a, b, c) = (q0, q1, q2)."""
    theta = np.asarray(theta, np.float64)
    inv_sqrt2 = 1.0 / np.sqrt(2.0)
    H = np.array([[1.0, 1.0], [1.0, -1.0]], np.complex128) * inv_sqrt2
    I2 = np.eye(2, dtype=np.complex128)

    def rx(t):
        c, s = np.cos(t / 2.0), np.sin(t / 2.0)
        return np.array([[c, -1j * s], [-1j * s, c]], np.complex128)

    A0 = rx(theta[0]) @ H
    A1 = rx(theta[1]) @ H
    G = np.kron(A0, np.kron(A1, I2))
    # CNOT control q0, target q2: out[a,b,c] = in[a,b,c^a]
    U = np.empty_like(G)
    for a in range(2):
        for b_ in range(2):
            for c in range(2):
                U[4 * a + 2 * b_ + c, :] = G[4 * a + 2 * b_ + (c ^ a), :]
    return U


def build_bd(theta):
    """Block-diagonal gate operands BDr/BDi, each [128, 256] fp32.

    Contraction layout: row = 8*w + k  (w in 0..15 state-within-group,
    k in 0..7 input component); col = 16*w2 + 2*k2 + ri (output component k2,
    ri = 0 real / 1 imag).  out = Tr'.T @ BDr + Ti'.T @ BDi."""
    U = circuit_unitary(theta)
    Ur = U.real.astype(np.float32)
    Ui = U.imag.astype(np.float32)
    BDr = np.zeros((128, 256), np.float32)
    BDi = np.zeros((128, 256), np.float32)
    for w in range(16):
        for k in range(8):
            row = 8 * w + k
            for k2 in range(8):
                col = 16 * w + 2 * k2
                BDr[row, col + 0] = Ur[k2, k]
                BDr[row, col + 1] = Ui[k2, k]
                BDi[row, col + 0] = -Ui[k2, k]
                BDi[row, col + 1] = Ur[k2, k]
    return BDr, BDi


def build_nc(n_iters=N_ITERS, w=W, use_fp32r=USE_FP32R):
    """Raw-bass pipeline (this walrus permits ONE sync-wait per instruction,
    which Tile's scheduler cannot guarantee — so semaphores are manual,
    with standalone wait_ge instructions).

    Per group g (2 groups/iter, 4096 states each):
      PE:  4 transposes -> tp[g%2] (PSUM), 4 matmuls tq(g-2) -> po[g%2]
      DVE: tq[g%4] <- tp[g%2]   (rounds to fp32r when enabled)
      ACT: ot half  <- po[g%2]; per iter: out-DMA (HWDGE/ACT ring)
      SP:  per iter: xr/xi in-DMAs
    """
    import contextlib

    nc = bass.Bass("TRN2", target_bir_lowering=False, debug=False)
    s = n_iters * 128 * w
    fw = w * 8
    ng = 2 * n_iters  # groups
    mm_dt = F32R if use_fp32r else F32

    xr = nc.dram_tensor("xr", [s, 8], F32, kind="ExternalInput").ap()
    xi = nc.dram_tensor("xi", [s, 8], F32, kind="ExternalInput").ap()
    bdr = nc.dram_tensor("bdr", [128, 256], mm_dt, kind="ExternalInput").ap()
    bdi = nc.dram_tensor("bdi", [128, 256], mm_dt, kind="ExternalInput").ap()
    idn = nc.dram_tensor("idn", [128, 128], F32, kind="ExternalInput").ap()
    out = nc.dram_tensor("out", [s, 16], F32, kind="ExternalOutput").ap()

    xr_v = xr.rearrange("(n p v) k -> n p (v k)", n=n_iters, p=128, v=w)
    xi_v = xi.rearrange("(n p v) k -> n p (v k)", n=n_iters, p=128, v=w)
    out_v = out.rearrange("(n p v) e -> n p (v e)", n=n_iters, p=128, v=w)

    with contextlib.ExitStack() as ctx:
        ent = ctx.enter_context
        block = ent(nc.Block())
        s_const = ent(nc.semaphore("s_const"))
        s_xr = [ent(nc.semaphore(f"s_xr{j}")) for j in range(3)]
        s_xi = [ent(nc.semaphore(f"s_xi{j}")) for j in range(3)]
        s_pe = ent(nc.semaphore("s_pe"))
        s_dve = ent(nc.semaphore("s_dve"))
        s_act = ent(nc.semaphore("s_act"))
        s_out = [ent(nc.semaphore(f"s_out{j}")) for j in range(3)]
        ident = ent(nc.sbuf_tensor("ident", [128, 128], F32))
        bdr_sb = ent(nc.sbuf_tensor("bdr_sb", [128, 256], mm_dt))
        bdi_sb = ent(nc.sbuf_tensor("bdi_sb", [128, 256], mm_dt))
        xr_sb = [ent(nc.sbuf_tensor(f"xr{j}", [128, fw], F32)) for j in range(3)]
        xi_sb = [ent(nc.sbuf_tensor(f"xi{j}", [128, fw], F32)) for j in range(3)]
        tq_sb = [ent(nc.sbuf_tensor(f"tq{j}", [128, 512], mm_dt)) for j in range(4)]
        ot_sb = [ent(nc.sbuf_tensor(f"ot{j}", [128, w * 16], F32)) for j in range(3)]
        tp_ps = [ent(nc.psum_tensor(f"tp{j}", [128, 512], F32)) for j in range(2)]
        po_ps = [ent(nc.psum_tensor(f"po{j}", [128, 512], F32)) for j in range(2)]

        # PE sem tick indices (see emission order in the tensor program)
        def pe_t(g):  # s_pe value once transposes of group g are done
            return max(1, 2 * g)

        def pe_m(g):  # s_pe value once matmuls of group g are done
            return 2 * ng if g == ng - 1 else 2 * g + 3

        @block.sync
        def _(sync):
            sync.dma_start(bdr_sb.ap(), bdr).then_inc(s_const, 16)
            sync.dma_start(bdi_sb.ap(), bdi).then_inc(s_const, 16)
            sync.dma_start(ident.ap(), idn).then_inc(s_const, 16)
            for i in range(n_iters):
                if i >= 3:
                    # xr/xi slots free once transposes of iter i-3 retired
                    sync.wait_ge(s_pe, pe_t(2 * (i - 3) + 1))
                sync.dma_start(xr_sb[i % 3].ap(), xr_v[i]).then_inc(s_xr[i % 3], 16)
                sync.dma_start(xi_sb[i % 3].ap(), xi_v[i]).then_inc(s_xi[i % 3], 16)

        @block.tensor
        def _(tensor):
            iap = ident.ap()

            def mms(g):
                tq = tq_sb[g % 4].ap()
                pp = po_ps[g % 2].ap()
                for q in range(2):
                    nc.tensor.matmul(
                        pp[:, 256 * q : 256 * q + 256],
                        tq[:, 256 * q : 256 * q + 128],
                        bdr_sb.ap(),
                        start=True,
                        stop=False,
                    )
                    mm = nc.tensor.matmul(
                        pp[:, 256 * q : 256 * q + 256],
                        tq[:, 256 * q + 128 : 256 * q + 256],
                        bdi_sb.ap(),
                        start=False,
                        stop=True,
                    )
                return mm

            tensor.wait_ge(s_const, 48)
            for g in range(ng):
                i, h = divmod(g, 2)
                if h == 0:
                    tensor.wait_ge(s_xr[i % 3], 16 * (i // 3 + 1))
                    tensor.wait_ge(s_xi[i % 3], 16 * (i // 3 + 1))
                if g >= 2:
                    # tp[g%2] free AND tq(g-2) ready (both = tcopy(g-2) done)
                    tensor.wait_ge(s_dve, g - 1)
                    if g >= 4:
                        # po[(g-2)%2] free: ocopy(g-4) done
                        tensor.wait_ge(s_act, g - 3)
                    mms(g - 2).then_inc(s_pe, 1)
                xs, ys = xr_sb[i % 3].ap(), xi_sb[i % 3].ap()
                tp = tp_ps[g % 2].ap()
                for j, b in enumerate((2 * h, 2 * h + 1)):
                    nc.tensor.transpose(
                        tp[:, 256 * j : 256 * j + 128],
                        xs[:, 128 * b : 128 * b + 128],
                        iap,
                    )
                    tr = nc.tensor.transpose(
                        tp[:, 256 * j + 128 : 256 * j + 256],
                        ys[:, 128 * b : 128 * b + 128],
                        iap,
                    )
                tr.then_inc(s_pe, 1)
            # drain the two pending matmul groups
            for g in (ng - 2, ng - 1):
                tensor.wait_ge(s_dve, g + 1)
                if g >= 2:
                    tensor.wait_ge(s_act, g - 1)
                mms(g).then_inc(s_pe, 1)

        @block.vector
        def _(vector):
            for g in range(ng):
                # transposes of g done; tq[g%4] free (matmuls g-4 done, older)
                vector.wait_ge(s_pe, max(pe_t(g), pe_m(g - 4) if g >= 4 else 0))
                nc.vector.tensor_copy(tq_sb[g % 4].ap(), tp_ps[g % 2].ap()).then_inc(
                    s_dve, 1
                )

        @block.scalar
        def _(scalar):
            for g in range(ng):
                i, h = divmod(g, 2)
                if h == 0 and i >= 3:
                    scalar.wait_ge(s_out[i % 3], 16 * (i // 3))
                scalar.wait_ge(s_pe, pe_m(g))
                nc.scalar.copy(
                    ot_sb[i % 3].ap()[:, 512 * h : 512 * h + 512], po_ps[g % 2].ap()
                ).then_inc(s_act, 1)
                if h == 1:
                    scalar.wait_ge(s_act, 2 * i + 2)
                    scalar.dma_start(out_v[i], ot_sb[i % 3].ap()).then_inc(
                        s_out[i % 3], 16
                    )

    return nc


def build_nc_tile(n_iters=N_ITERS, w=W, use_fp32r=USE_FP32R):
    """One-core Bass module; same NEFF runs SPMD on all 8 cores."""
    nc = bass.Bass("TRN2", target_bir_lowering=False, debug=False)
    s = n_iters * 128 * w
    fw = w * 8  # floats per partition-row of one input tile
    nb = (w * 8) // 128  # 128-col sub-blocks per iteration

    mm_dt = F32R if use_fp32r else F32

    xr = nc.dram_tensor("xr", [s, 8], F32, kind="ExternalInput").ap()
    xi = nc.dram_tensor("xi", [s, 8], F32, kind="ExternalInput").ap()
    bdr = nc.dram_tensor("bdr", [128, 256], mm_dt, kind="ExternalInput").ap()
    bdi = nc.dram_tensor("bdi", [128, 256], mm_dt, kind="ExternalInput").ap()
    out = nc.dram_tensor("out", [s, 16], F32, kind="ExternalOutput").ap()

    xr_v = xr.rearrange("(n p v) k -> n p (v k)", n=n_iters, p=128, v=w)
    xi_v = xi.rearrange("(n p v) k -> n p (v k)", n=n_iters, p=128, v=w)
    out_v = out.rearrange("(n p v) e -> n p (v e)", n=n_iters, p=128, v=w)

    with tile.TileContext(nc) as tc:
        with (
            tc.tile_pool(name="const", bufs=1) as const,
            tc.tile_pool(name="inp", bufs=3) as inpool,
            tc.tile_pool(name="tsb", bufs=4) as tpool,
            tc.tile_pool(name="osb", bufs=3) as opool,
            tc.tile_pool(name="pt", bufs=3, space="PSUM") as pt,
            tc.tile_pool(name="po", bufs=4, space="PSUM") as po,
            tc.tile_pool(name="ps", bufs=1, space="PSUM") as ps,
        ):
            ident = const.tile([128, 128], F32)
            masks.make_identity(nc, ident[:])
            bdr_sb = const.tile([128, 256], mm_dt)
            nc.sync.dma_start(bdr_sb[:], bdr)
            bdi_sb = const.tile([128, 256], mm_dt)
            nc.sync.dma_start(bdi_sb[:], bdi)

            # The PE matmult/transpose instructions lower to a fused-LDWEIGHTS
            # form whose sync struct fits only ONE wait command (walrus
            # "Too many sync wait commands").  Tiny 32x32 PE "absorber"
            # transposes pre-observe semaphores so every real PE op carries at
            # most one wait.  PE-internal ordering needs no waits, so a WAW
            # touch of the target PSUM region forces absorber-before-real-op.
            scr = ps.tile([128, 32], F32)  # scratch PSUM bank for absorbers
            i32 = ident[0:32, 0:32]

            def absorb(read_ap=None):
                nc.tensor.transpose(scr[0:32, 0:32], read_ap or i32, i32)

            absorb()  # observes Pool (identity ready)
            absorb(bdr_sb[0:32, 0:32].bitcast(F32))  # observes bdr DMA
            absorb(bdi_sb[0:32, 0:32].bitcast(F32))  # observes bdi DMA

            for i in range(n_iters):
                xr_t = inpool.tile([128, fw], F32, tag="xr")
                nc.sync.dma_start(xr_t[:], xr_v[i])
                xi_t = inpool.tile([128, fw], F32, tag="xi")
                nc.sync.dma_start(xi_t[:], xi_v[i])

                o_t = opool.tile([128, w * 16], F32)
                # ACT absorber: first touch of the o_t slot eats the
                # out-DMA slot-release wait so real ACT copies wait PE-only
                nc.scalar.copy(o_t[0:1, 0:1], ident[0:1, 0:1])
                for h in range(nb // 2):
                    bA, bB = 2 * h, 2 * h + 1
                    p_t = pt.tile([128, 512], F32)
                    # PE absorber: eat p_t slot-release (ACT t-copy)
                    nc.tensor.transpose(p_t[0:32, 0:32], i32, i32)
                    nc.tensor.transpose(
                        p_t[:, 0:128], xr_t[:, 128 * bA : 128 * bA + 128], ident[:]
                    )
                    nc.tensor.transpose(
                        p_t[:, 128:256], xi_t[:, 128 * bA : 128 * bA + 128], ident[:]
                    )
                    nc.tensor.transpose(
                        p_t[:, 256:384], xr_t[:, 128 * bB : 128 * bB + 128], ident[:]
                    )
                    nc.tensor.transpose(
                        p_t[:, 384:512], xi_t[:, 128 * bB : 128 * bB + 128], ident[:]
                    )
                    t_sb = tpool.tile([128, 512], mm_dt)
                    nc.scalar.copy(t_sb[:], p_t[:])
                    p_o = po.tile([128, 512], F32)
                    # PE absorber: eat p_o slot-release (ACT o-copy)
                    nc.tensor.transpose(p_o[0:32, 0:32], i32, i32)
                    for q, bq in enumerate((bA, bB)):
                        nc.tensor.matmul(
                            p_o[:, 256 * q : 256 * q + 256],
                            t_sb[:, 256 * q : 256 * q + 128],
                            bdr_sb[:],
                            start=True,
                            stop=False,
                        )
                        nc.tensor.matmul(
                            p_o[:, 256 * q : 256 * q + 256],
                            t_sb[:, 256 * q + 128 : 256 * q + 256],
                            bdi_sb[:],
                            start=False,
                            stop=True,
                        )
                    nc.scalar.copy(o_t[:, 512 * h : 512 * h + 512], p_o[:])

                nc.scalar.dma_start(out_v[i], o_t[:])
    return nc


def build_stag(theta):
    """[128, 128] fp32 staging tile for the on-chip block-diagonal build.

    Cols 0:64 hold a 32x64 block-diagonal arrangement of 4 BDr 8x16 gate
    blocks (block a at rows 8a, cols 16a), replicated 4x down the
    partitions; cols 64:128 the same for BDi.  On-chip, DVE does 8 aligned
    copies: bd{r,i}[32q:32q+32, 64q:64q+64] <- stag[32q:32q+32, ...], which
    lands block w=4q+a at rows 8w, cols 16w of the [128, 256] operand
    (engine APs must start at 32-aligned partitions, hence this layout)."""
    BDr, BDi = build_bd(theta)
    base = np.zeros((32, 128), np.float32)
    for a in range(4):
        base[8 * a : 8 * a + 8, 16 * a : 16 * a + 16] = BDr[0:8, 0:16]
        base[8 * a : 8 * a + 8, 64 + 16 * a : 64 + 16 * a + 16] = BDi[0:8, 0:16]
    blocks = np.tile(base, (4, 1))
    # cols 128:256: identity, delivered free by the same staging DMA (v3
    # slices it as the PE transpose operand; v2 ignores it)
    return np.ascontiguousarray(
        np.concatenate([blocks, np.eye(128, dtype=np.float32)], axis=1)
    )


def build_nc_v2(n_iters=N_ITERS, w=W, use_fp32r=USE_FP32R):
    """DMA-roofline-tuned raw-bass pipeline (96598ns vs 102777ns for v1).

    Every DMA transfer serializes through one exclusive DMA-engine pool at
    360 GB/s, so mandatory traffic (16.8MB in + 16.8MB out per core) sets a
    ~93.2us floor.  v2 closes the gaps the baseline had on top of that:
      - 6-deep input buffering: the in-DMA stream never stalls on the PE
        p-state cold start (v1 lost ~2.6us there).
      - constants cost 182ns of DMA instead of ~910ns: identity is built
        on GpSimd (memset+affine_select), the gate matrices from a
        [128,128] staging DMA (issued via GpSimd SWDGE: no HWDGE
        contention) + 8 aligned DVE copies into the block-diagonal
        positions (GpSimd pre-zeroes the targets).
      - 6-deep output buffering so the out-DMA stream lags compute and
        keeps the DMA pool saturated through the drain (v1 lost ~2.8us);
        the last iteration ships as two half-tiles to shorten the final
        matmul->copy->DMA chain.
    Remaining overhead vs the byte floor is structural: ~1.0us framework
    prologue + 1.3us first-DMA issue path + 900ns final DMA-completion
    semaphore propagation.

    Correctness notes (all verified by CoreSim's race detector + real HW):
      - GpSimd instructions execute asynchronously across DSP cores, so
        RAW deps between consecutive GpSimd ops need semaphore chaining.
      - DMA completions on different in-flight transfers are unordered:
        completion sems must be per-buffer-slot (values disambiguated by
        the slot-reuse causal chain), not one shared counter.
      - Engine APs must start at 32-aligned partitions; data written by
        engine ops into fp32r matmul operands must be written as fp32r.
    Per-group dataflow is unchanged from v1 (PE transpose -> DVE fp32r
    copy -> PE matmul vs block-diag gates -> ACT copy -> DMA out).
    """
    import contextlib

    nc = bass.Bass("TRN2", target_bir_lowering=False, debug=False)
    s = n_iters * 128 * w
    fw = w * 8
    ng = 2 * n_iters
    mm_dt = F32R if use_fp32r else F32
    NIN, NOT, NTQ, NPO, NTP = 6, 6, 4, 3, 2

    xr = nc.dram_tensor("xr", [s, 8], F32, kind="ExternalInput").ap()
    xi = nc.dram_tensor("xi", [s, 8], F32, kind="ExternalInput").ap()
    stag = nc.dram_tensor("stag", [128, 256], F32, kind="ExternalInput").ap()
    out = nc.dram_tensor("out", [s, 16], F32, kind="ExternalOutput").ap()

    xr_v = xr.rearrange("(n p v) k -> n p (v k)", n=n_iters, p=128, v=w)
    xi_v = xi.rearrange("(n p v) k -> n p (v k)", n=n_iters, p=128, v=w)
    out_v = out.rearrange("(n p v) e -> n p (v e)", n=n_iters, p=128, v=w)

    # PE instruction stream: transposes of group g, then matmuls of g-2.
    # tick[op] = s_pe value once that group-op's final instruction retires.
    stream = []
    for g in range(ng):
        stream.append(("t", g))
        if g >= 2:
            stream.append(("m", g - 2))
    stream.append(("m", ng - 2))
    stream.append(("m", ng - 1))
    tick = {op: n + 1 for n, op in enumerate(stream)}

    with contextlib.ExitStack() as ctx:
        ent = ctx.enter_context
        block = ent(nc.Block())
        s_cb = ent(nc.semaphore("s_cb"))    # staging DMA done
        s_idn = ent(nc.semaphore("s_idn"))  # identity built (GpSimd)
        s_g = ent(nc.semaphore("s_g"))      # GpSimd intra-engine ordering
        s_z = ent(nc.semaphore("s_z"))      # bd targets zeroed (GpSimd)
        s_bd = ent(nc.semaphore("s_bd"))    # gate matrices built (DVE)
        # per-slot DMA-completion sems: a slot-sem's increments are causally
        # ordered by the slot-reuse chain, so wait values are unambiguous
        # (concurrent DMA completions on one shared counter are not)
        s_xr = [ent(nc.semaphore(f"s_xr{j}")) for j in range(NIN)]
        s_xi = [ent(nc.semaphore(f"s_xi{j}")) for j in range(NIN)]
        s_o = [ent(nc.semaphore(f"s_o{j}")) for j in range(NOT)]
        s_pe = ent(nc.semaphore("s_pe"))    # PE group ticks
        s_dve = ent(nc.semaphore("s_dve"))  # tq copies done
        s_act = ent(nc.semaphore("s_act"))  # ACT po->ot copies done

        ident = ent(nc.sbuf_tensor("ident", [128, 128], F32))
        stag_sb = ent(nc.sbuf_tensor("stag_sb", [128, 256], F32))
        bdr_sb = ent(nc.sbuf_tensor("bdr_sb", [128, 256], mm_dt))
        bdi_sb = ent(nc.sbuf_tensor("bdi_sb", [128, 256], mm_dt))
        xr_sb = [ent(nc.sbuf_tensor(f"xr{j}", [128, fw], F32)) for j in range(NIN)]
        xi_sb = [ent(nc.sbuf_tensor(f"xi{j}", [128, fw], F32)) for j in range(NIN)]
        tq_sb = [ent(nc.sbuf_tensor(f"tq{j}", [128, 512], mm_dt)) for j in range(NTQ)]
        ot_sb = [ent(nc.sbuf_tensor(f"ot{j}", [128, w * 16], F32)) for j in range(NOT)]
        tp_ps = [ent(nc.psum_tensor(f"tp{j}", [128, 512], F32)) for j in range(NTP)]
        po_ps = [ent(nc.psum_tensor(f"po{j}", [128, 512], F32)) for j in range(NPO)]

        @block.sync
        def _(sync):
            for i in range(n_iters):
                if i >= NIN:
                    # slot free once transposes of iter i-NIN retired
                    sync.wait_ge(s_pe, tick[("t", 2 * (i - NIN) + 1)])
                sync.dma_start(xr_sb[i % NIN].ap(), xr_v[i]).then_inc(
                    s_xr[i % NIN], 16
                )
                sync.dma_start(xi_sb[i % NIN].ap(), xi_v[i]).then_inc(
                    s_xi[i % NIN], 16
                )

        @block.gpsimd
        def _(gp):
            # SWDGE path: no HWDGE contention with SP's input-DMA stream
            nc.gpsimd.dma_start(stag_sb.ap(), stag).then_inc(s_cb, 16)
            # GpSimd ops execute asynchronously across DSP cores: consecutive
            # instructions are NOT ordered, so chain RAW deps through s_g
            nc.gpsimd.memset(ident.ap(), 0.0).then_inc(s_g, 1)
            gp.wait_ge(s_g, 1)
            nc.gpsimd.affine_select(
                out=ident.ap(),
                in_=ident.ap(),
                compare_op=mybir.AluOpType.not_equal,
                fill=1.0,
                base=0,
                pattern=[[-1, 128]],
                channel_multiplier=1,
            ).then_inc(s_idn, 1)
            # the two bd memsets are mutually independent; each bumps s_z and
            # the DVE scatter waits for both
            nc.gpsimd.memset(bdr_sb.ap().bitcast(F32), 0.0).then_inc(s_z, 1)
            nc.gpsimd.memset(bdi_sb.ap().bitcast(F32), 0.0).then_inc(s_z, 1)

        @block.vector
        def _(vector):
            vector.wait_ge(s_cb, 16)
            nc.vector.tensor_copy(
                ident.ap(), stag_sb.ap()[0:128, 128:256]
            ).then_inc(s_idn, 1)
            vector.wait_ge(s_z, 2)
            last = None
            for q in range(4):
                sl = slice(32 * q, 32 * q + 32)
                cl = slice(64 * q, 64 * q + 64)
                nc.vector.tensor_copy(
                    bdr_sb.ap()[sl, cl], stag_sb.ap()[sl, 0:64]
                )
                last = nc.vector.tensor_copy(
                    bdi_sb.ap()[sl, cl], stag_sb.ap()[sl, 64:128]
                )
            last.then_inc(s_bd, 1)
            for g in range(ng):
                # transposes g done (also implies matmuls g-NTQ done: that
                # op precedes ("t", g) in the PE stream, so tq slot is free)
                vector.wait_ge(s_pe, tick[("t", g)])
                nc.vector.tensor_copy(
                    tq_sb[g % NTQ].ap(), tp_ps[g % NTP].ap()
                ).then_inc(s_dve, 1)

        @block.tensor
        def _(tensor):
            iap = ident.ap()

            def emit_t(g):
                i, h = divmod(g, 2)
                if h == 0:
                    tensor.wait_ge(s_xr[i % NIN], 16 * (i // NIN + 1))
                    tensor.wait_ge(s_xi[i % NIN], 16 * (i // NIN + 1))
                if g == 0:
                    tensor.wait_ge(s_idn, 1)
                if g >= NTP:
                    # tp slot free once DVE copied group g-NTP out
                    tensor.wait_ge(s_dve, g - NTP + 1)
                xs, ys = xr_sb[i % NIN].ap(), xi_sb[i % NIN].ap()
                tp = tp_ps[g % NTP].ap()
                tr = None
                for j, b in enumerate((2 * h, 2 * h + 1)):
                    nc.tensor.transpose(
                        tp[:, 256 * j : 256 * j + 128],
                        xs[:, 128 * b : 128 * b + 128],
                        iap,
                    )
                    tr = nc.tensor.transpose(
                        tp[:, 256 * j + 128 : 256 * j + 256],
                        ys[:, 128 * b : 128 * b + 128],
                        iap,
                    )
                tr.then_inc(s_pe, 1)

            def emit_m(g, explicit_dve_wait):
                if g == 0:
                    tensor.wait_ge(s_bd, 1)
                if explicit_dve_wait:
                    # tq g ready (in the main loop the preceding emit_t(g+2)
                    # already waited for s_dve >= g+1)
                    tensor.wait_ge(s_dve, g + 1)
                if g >= NPO:
                    # po slot free once ACT copied group g-NPO out
                    tensor.wait_ge(s_act, g - NPO + 1)
                tq = tq_sb[g % NTQ].ap()
                pp = po_ps[g % NPO].ap()
                mm = None
                for q in range(2):
                    nc.tensor.matmul(
                        pp[:, 256 * q : 256 * q + 256],
                        tq[:, 256 * q : 256 * q + 128],
                        bdr_sb.ap(),
                        start=True,
                        stop=False,
                    )
                    mm = nc.tensor.matmul(
                        pp[:, 256 * q : 256 * q + 256],
                        tq[:, 256 * q + 128 : 256 * q + 256],
                        bdi_sb.ap(),
                        start=False,
                        stop=True,
                    )
                mm.then_inc(s_pe, 1)

            for g in range(ng):
                emit_t(g)
                if g >= 2:
                    emit_m(g - 2, explicit_dve_wait=False)
            emit_m(ng - 2, explicit_dve_wait=True)
            emit_m(ng - 1, explicit_dve_wait=True)

        @block.scalar
        def _(scalar):
            for i in range(n_iters):
                last_iter = i == n_iters - 1
                for h in (0, 1):
                    g = 2 * i + h
                    if h == 0 and i >= NOT:
                        # ot slot free once iter i-NOT's out-DMA completed
                        scalar.wait_ge(s_o[i % NOT], 16 * (i // NOT))
                    scalar.wait_ge(s_pe, tick[("m", g)])
                    nc.scalar.copy(
                        ot_sb[i % NOT].ap()[:, 512 * h : 512 * h + 512],
                        po_ps[g % NPO].ap(),
                    ).then_inc(s_act, 1)
                    if last_iter:
                        # split the final tile: each half ships as soon as
                        # its ACT copy lands, halving the drain granule
                        scalar.wait_ge(s_act, g + 1)
                        scalar.dma_start(
                            out_v[i][:, 512 * h : 512 * h + 512],
                            ot_sb[i % NOT].ap()[:, 512 * h : 512 * h + 512],
                        ).then_inc(s_o[i % NOT], 16)
                if not last_iter:
                    scalar.wait_ge(s_act, 2 * i + 2)
                    scalar.dma_start(out_v[i], ot_sb[i % NOT].ap()).then_inc(
                        s_o[i % NOT], 16
                    )

    return nc


F16 = mybir.dt.float16


def build_nc_v3(n_iters=8, w=256, out_dt=F32):
    """fp16-load pipeline: ~73us modeled vs 96.6us for v2.

    GpSimd's software DGE is the only DMA path that can cast dtypes, and a
    casting DMA's transfer time is charged on the OUTPUT bytes - so loading
    x_real/x_imag as fp32->fp16 halves the input stream's DMA-engine time
    (46.6us -> 23.3us).  fp16 also makes PE transposes 2x faster and keeps
    matmuls at 1 cycle/row.  Accumulation stays fp32 in PSUM and the output
    ships as fp32, so only inputs and gates are rounded (~1e-3 rel err vs
    the 2e-2 gate).  W=256 states/partition-row per tile keeps the Pool
    engine's SWDGE descriptor generation (~1us per DMA) well off the
    critical path and amortizes instruction overheads.
    """
    import contextlib

    nc = bass.Bass("TRN2", target_bir_lowering=False, debug=False)
    s = n_iters * 128 * w
    fw = w * 8            # fp16 elems per partition-row of one input tile
    GPI = w // 32         # groups (2x 128-col blocks) per iteration
    ng = GPI * n_iters
    NIN, NOT, NTQ, NPO, NTP = 6, 4, 9, 3, 2

    xr = nc.dram_tensor("xr", [s, 8], F32, kind="ExternalInput").ap()
    xi = nc.dram_tensor("xi", [s, 8], F32, kind="ExternalInput").ap()
    stag = nc.dram_tensor("stag", [128, 256], F32, kind="ExternalInput").ap()
    out = nc.dram_tensor("out", [s, 16], out_dt, kind="ExternalOutput").ap()

    xr_v = xr.rearrange("(n p v) k -> n p (v k)", n=n_iters, p=128, v=w)
    xi_v = xi.rearrange("(n p v) k -> n p (v k)", n=n_iters, p=128, v=w)
    out_v = out.rearrange("(n p v) e -> n p (v e)", n=n_iters, p=128, v=w)

    # PE stream: matmuls lag transposes by LAG groups so the late-arriving
    # gate matrices don't stall the in-order PE queue (transposes recycle
    # input slots, so they must keep flowing)
    LAG = 8
    stream = []
    for g in range(ng):
        stream.append(("t", g))
        if g >= LAG:
            stream.append(("m", g - LAG))
    for g in range(ng - LAG, ng):
        stream.append(("m", g))
    tick = {op: n + 1 for n, op in enumerate(stream)}

    with contextlib.ExitStack() as ctx:
        ent = ctx.enter_context
        block = ent(nc.Block())
        s_cb = ent(nc.semaphore("s_cb"))
        s_idn = ent(nc.semaphore("s_idn"))
        s_z = ent(nc.semaphore("s_z"))
        s_bd = ent(nc.semaphore("s_bd"))
        s_xr = [ent(nc.semaphore(f"s_xr{j}")) for j in range(NIN)]
        s_xi = [ent(nc.semaphore(f"s_xi{j}")) for j in range(NIN)]
        s_o = [ent(nc.semaphore(f"s_o{j}")) for j in range(NOT)]
        s_pe = ent(nc.semaphore("s_pe"))
        s_dve = ent(nc.semaphore("s_dve"))
        s_act = ent(nc.semaphore("s_act"))

        stag_sb = ent(nc.sbuf_tensor("stag_sb", [128, 256], F32))
        ident = ent(nc.sbuf_tensor("ident", [128, 128], F16))
        bdr_sb = ent(nc.sbuf_tensor("bdr_sb", [128, 256], F16))
        bdi_sb = ent(nc.sbuf_tensor("bdi_sb", [128, 256], F16))
        xr_sb = [ent(nc.sbuf_tensor(f"xr{j}", [128, fw], F16)) for j in range(NIN)]
        xi_sb = [ent(nc.sbuf_tensor(f"xi{j}", [128, fw], F16)) for j in range(NIN)]
        tq_sb = [ent(nc.sbuf_tensor(f"tq{j}", [128, 512], F16)) for j in range(NTQ)]
        ot_sb = [
            ent(nc.sbuf_tensor(f"ot{j}", [128, w * 16], out_dt)) for j in range(NOT)
        ]
        tp_ps = [ent(nc.psum_tensor(f"tp{j}", [128, 512], F16)) for j in range(NTP)]
        po_ps = [ent(nc.psum_tensor(f"po{j}", [128, 512], F32)) for j in range(NPO)]

        @block.sync
        def _(sync):
            # SP's HWDGE path issues ~380ns before Pool's first SWDGE DMA
            # can; the staging transfer fills that otherwise-dead window
            sync.dma_start(stag_sb.ap(), stag).then_inc(s_cb, 16)
        @block.gpsimd
        def _(gp):
            # all input DMAs ride the software DGE (only path that casts);
            # descriptor generation (~1us/DMA) runs on the otherwise-idle
            # Pool engine, overlapped with the transfers themselves
            for i in range(n_iters):
                if i >= NIN:
                    gp.wait_ge(s_pe, tick[("t", GPI * (i - NIN) + GPI - 1)])
                nc.gpsimd.dma_start(xr_sb[i % NIN].ap(), xr_v[i]).then_inc(
                    s_xr[i % NIN], 16
                )
                nc.gpsimd.dma_start(xi_sb[i % NIN].ap(), xi_v[i]).then_inc(
                    s_xi[i % NIN], 16
                )
                if i == min(2, n_iters - 1):
                    # bd zero-fills slot in here: the LAG matmul lag means a
                    # late s_bd only delays the out stream (covered by the
                    # input-transfer backlog), and xr3's descriptor
                    # generation still beats the DMA pool's demand
                    nc.gpsimd.memset(bdr_sb.ap(), 0.0).then_inc(s_z, 1)
                    nc.gpsimd.memset(bdi_sb.ap(), 0.0).then_inc(s_z, 1)

        @block.vector
        def _(vector):
            vector.wait_ge(s_cb, 16)
            nc.vector.tensor_copy(
                ident.ap(), stag_sb.ap()[0:128, 128:256]
            ).then_inc(s_idn, 1)
            vector.wait_ge(s_z, 2)
            last = None
            for q in range(4):
                sl = slice(32 * q, 32 * q + 32)
                cl = slice(64 * q, 64 * q + 64)
                nc.vector.tensor_copy(bdr_sb.ap()[sl, cl], stag_sb.ap()[sl, 0:64])
                last = nc.vector.tensor_copy(
                    bdi_sb.ap()[sl, cl], stag_sb.ap()[sl, 64:128]
                )
            last.then_inc(s_bd, 1)
            for g in range(ng):
                vector.wait_ge(s_pe, tick[("t", g)])
                nc.vector.tensor_copy(
                    tq_sb[g % NTQ].ap(), tp_ps[g % NTP].ap()
                ).then_inc(s_dve, 1)

        @block.tensor
        def _(tensor):
            iap = ident.ap()

            def emit_t(g):
                i, h = divmod(g, GPI)
                if h == 0:
                    tensor.wait_ge(s_xr[i % NIN], 16 * (i // NIN + 1))
                    tensor.wait_ge(s_xi[i % NIN], 16 * (i // NIN + 1))
                if g == 0:
                    # identity: staging DMA cols 128:256, f16'd by DVE
                    tensor.wait_ge(s_idn, 1)
                if g >= NTP:
                    tensor.wait_ge(s_dve, g - NTP + 1)
                xs, ys = xr_sb[i % NIN].ap(), xi_sb[i % NIN].ap()
                tp = tp_ps[g % NTP].ap()
                tr = None
                for j, b in enumerate((2 * h, 2 * h + 1)):
                    nc.tensor.transpose(
                        tp[:, 256 * j : 256 * j + 128],
                        xs[:, 128 * b : 128 * b + 128],
                        iap,
                    )
                    tr = nc.tensor.transpose(
                        tp[:, 256 * j + 128 : 256 * j + 256],
                        ys[:, 128 * b : 128 * b + 128],
                        iap,
                    )
                tr.then_inc(s_pe, 1)

            def emit_m(g, explicit_dve_wait):
                if g == 0:
                    tensor.wait_ge(s_bd, 1)
                if explicit_dve_wait:
                    tensor.wait_ge(s_dve, g + 1)
                if g >= NPO:
                    tensor.wait_ge(s_act, g - NPO + 1)
                tq = tq_sb[g % NTQ].ap()
                pp = po_ps[g % NPO].ap()
                mm = None
                for q in range(2):
                    nc.tensor.matmul(
                        pp[:, 256 * q : 256 * q + 256],
                        tq[:, 256 * q : 256 * q + 128],
                        bdr_sb.ap(),
                        start=True,
                        stop=False,
                    )
                    mm = nc.tensor.matmul(
                        pp[:, 256 * q : 256 * q + 256],
                        tq[:, 256 * q + 128 : 256 * q + 256],
                        bdi_sb.ap(),
                        start=False,
                        stop=True,
                    )
                mm.then_inc(s_pe, 1)

            for g in range(ng):
                emit_t(g)
                if g >= LAG:
                    emit_m(g - LAG, explicit_dve_wait=False)
            for g in range(ng - LAG, ng):
                emit_m(g, explicit_dve_wait=True)

        @block.scalar
        def _(scalar):
            for i in range(n_iters):
                last_iter = i == n_iters - 1
                for h in range(GPI):
                    g = GPI * i + h
                    if h == 0 and i >= NOT:
                        scalar.wait_ge(s_o[i % NOT], 16 * (i // NOT))
                    scalar.wait_ge(s_pe, tick[("m", g)])
                    nc.scalar.copy(
                        ot_sb[i % NOT].ap()[:, 512 * h : 512 * h + 512],
                        po_ps[g % NPO].ap(),
                    ).then_inc(s_act, 1)
                    if last_iter:
                        scalar.wait_ge(s_act, g + 1)
                        scalar.dma_start(
                            out_v[i][:, 512 * h : 512 * h + 512],
                            ot_sb[i % NOT].ap()[:, 512 * h : 512 * h + 512],
                        ).then_inc(s_o[i % NOT], 16)
                if not last_iter:
                    scalar.wait_ge(s_act, GPI * (i + 1))
                    scalar.dma_start(out_v[i], ot_sb[i % NOT].ap()).then_inc(
                        s_o[i % NOT], 16
                    )

    return nc


_NC_CACHE = {}

KERNEL_V = os.environ.get("KERNEL_V", "4")


def _get_nc(n_iters, w, use_fp32r):
    key = (KERNEL_V, n_iters, w, use_fp32r)
    if key not in _NC_CACHE:
        if KERNEL_V == "4":
            # v4 = v3 + fp16 output (PSUM fp32 -> ACT cast -> fp16 DMA out,
            # host casts back to fp32): halves the out-DMA stream
            _NC_CACHE[key] = build_nc_v3(out_dt=F16)
        elif KERNEL_V == "3":
            # v3 fixes its own tiling (8 iters x 256 states/partition-row)
            _NC_CACHE[key] = build_nc_v3()
        elif KERNEL_V == "2":
            _NC_CACHE[key] = build_nc_v2(n_iters, w, use_fp32r)
        else:
            _NC_CACHE[key] = build_nc(n_iters, w, use_fp32r)
    return _NC_CACHE[key]


def kernel(x_real, x_imag, theta, angle=None, **_unused):
    x_real = np.ascontiguousarray(np.asarray(x_real, np.float32))
    x_imag = np.ascontiguousarray(np.asarray(x_imag, np.float32))
    theta = np.asarray(theta, np.float32)
    assert x_real.shape == (B, 8), x_real.shape

    nc = _get_nc(N_ITERS, W, USE_FP32R)

    if KERNEL_V in ("2", "3", "4"):
        stag_np = build_stag(theta)
        consts = {"stag": stag_np}
    else:
        BDr, BDi = build_bd(theta)
        consts = {"bdr": BDr, "bdi": BDi, "idn": np.eye(128, dtype=np.float32)}

    in_maps = []
    for c in range(N_CORES):
        sl = slice(c * S_CORE, (c + 1) * S_CORE)
        in_maps.append({"xr": x_real[sl], "xi": x_imag[sl], **consts})

    res = run_bass_kernel_spmd(nc, in_maps, core_ids=list(range(N_CORES)))
    out = np.concatenate([r["out"] for r in res.results], axis=0)
    if out.dtype != np.float32:
        out = out.astype(np.float32)
    return out.reshape(B, 8, 2)

